# revision 56
# baseline (speedup 1.0000x reference)
"""Tensor-parallel multi-head attention for Trainium2 (8 NeuronCores).

Problem: nn_MultiHeadAttention (B=2, N=2048, C=1024, H=16, D=64), fp32.

Sharding: core = batch * 4 + head_group; each core handles 1 batch and 4
heads (tensor-parallel over heads, data-parallel over batch). Each core
computes its heads' QKV projections, attention, and a *partial* output
projection (its 256 rows of w_proj); the host sums the 4 partials per
batch and adds b_proj.

Kernel math notes:
  - x is transposed and block-laid-out on the host so every input DMA is
    a contiguous >=256KB read with >=2KB per-partition lines; all matmul
    operands are bf16 (fp32 PSUM accumulation), end-to-end rel err ~6e-3.
  - Scores are computed transposed: sT[m, n] = k[m]·q[n] with keys m on
    partitions -- so P@V needs no on-chip transposes.
  - Softmax: no max-subtraction (exp in fp32 is safe); denominator via a
    ones-column appended to V (row 64 of the attention accumulator);
    normalization by reciprocal + DMA partition-broadcast multiply.
  - k-bias is softmax-invariant and dropped; v-bias is added to the
    attention output; q-bias at QKV eviction; proj-bias on the host.
  - A dozen dependency-free warm-up matmuls flip the PE's HAM clock gate
    to full rate while the inputs stream in.
"""

import numpy as np
from contextlib import ExitStack

P = 128
C = 1024
D = 64
N_CORES = 8

_BUILT = {}
TRACE = False   # set True (e.g. from test.py) to capture an NTFF profile
LAST_RESULTS = None  # BassKernelResults of the most recent kernel() call


def _build(n_tok, debug=False):
    import concourse.bass as bass
    import concourse.mybir as mybir
    import concourse.tile as tile
    from concourse import bacc
    from concourse.bass import ts

    fp32 = mybir.dt.float32
    bf16 = mybir.dt.bfloat16
    Exp = mybir.ActivationFunctionType.Exp
    mult = mybir.AluOpType.mult

    NC5 = n_tok // 512  # 512-wide query chunks
    MC = n_tok // 128   # 128-wide key chunks
    CC = C // P         # contraction chunks for projections
    NH = n_tok // 1024  # token halves (DMA granularity)

    nc = bacc.Bacc("TRN2", target_bir_lowering=False, debug=debug)

    xt_d = nc.dram_tensor(
        "xt", [NH, CC, P, 1024], bf16, kind="ExternalInput"
    ).ap()
    wqkk_d = nc.dram_tensor("w_qkk", [2, P, 1024], bf16, kind="ExternalInput").ap()
    wqkq_d = nc.dram_tensor("w_qkq", [2, P, 1024], bf16, kind="ExternalInput").ap()
    wv_d = nc.dram_tensor("w_v", [2, P, 1024], bf16, kind="ExternalInput").ap()
    wp_d = nc.dram_tensor("w_p", [256, C], bf16, kind="ExternalInput").ap()
    bq_d = nc.dram_tensor("b_q", [256], fp32, kind="ExternalInput").ap()
    bv_d = nc.dram_tensor("b_v2", [64, 4], fp32, kind="ExternalInput").ap()
    out_d = nc.dram_tensor("out", [n_tok, C], bf16, kind="ExternalOutput").ap()

    with tile.TileContext(nc) as tc, ExitStack() as ctx:
        persist = ctx.enter_context(tc.tile_pool(name="persist", bufs=1))
        p_pool = ctx.enter_context(tc.tile_pool(name="p_pool", bufs=4))
        ev_pool = ctx.enter_context(tc.tile_pool(name="ev_pool", bufs=2))
        ph1_cm = tc.tile_pool(name="ph1", bufs=1)
        ph1 = ph1_cm.__enter__()
        s_pool = ctx.enter_context(tc.tile_pool(name="s", bufs=2, space="PSUM"))
        sm_pool = ctx.enter_context(tc.tile_pool(name="sm", bufs=2, space="PSUM"))
        o2_pool = ctx.enter_context(tc.tile_pool(name="o2", bufs=2, space="PSUM"))
        dram_pool = ctx.enter_context(tc.tile_pool(name="dram", bufs=4, space="DRAM"))

        xt = ph1.tile([P, CC, n_tok], bf16)
        wqkk = ph1.tile([P, 2, 1024], bf16)  # k-columns, contraction quads
        wqkq = ph1.tile([P, 2, 1024], bf16)  # q-columns, contraction quads
        wv = ph1.tile([P, 2, 1024], bf16)    # contraction-chunk quads
        bq = ph1.tile([P, 2], fp32)
        wp = persist.tile([P, 2, C], bf16)
        bv = persist.tile([64, 4], fp32)
        qk = persist.tile([P, 4, n_tok], bf16)   # jc: 0,1 = qT pairs, 2,3 = kT
        vsb = persist.tile([P, MC, 4, 65], bf16)  # token-major V + ones column
        o2n = persist.tile([P, 2, n_tok], bf16)   # normalized attn out

        # three DMA queues in parallel; the k(0)/q(0)-critical set (qk
        # weights + x token-half 0) is split across the queues and leads
        # each one, so HBM bandwidth goes to first-needed data
        def xd(h, cc):
            return lambda q: q.dma_start(xt[:, cc, ts(h, 1024)], xt_d[h, cc])
        dma_plan = {
            "scalar": [
                lambda q: q.dma_start(bq[:], bq_d.rearrange("(c p) -> p c", p=P)),
                lambda q: q.dma_start(bv[:], bv_d),
                lambda q: q.dma_start(wqkk[:, 1], wqkk_d[1]),
                xd(0, 0), xd(0, 3),
                lambda q: q.dma_start(wqkq[:, 0], wqkq_d[0]),
                xd(1, 0), xd(1, 3),
            ],
            "sync": [
                xd(0, 1), xd(0, 4), xd(0, 7),
                lambda q: q.dma_start(wqkq[:, 1], wqkq_d[1]),
                xd(1, 1), xd(1, 4), xd(1, 7),
                lambda q: q.dma_start(
                    wp[:], wp_d.rearrange("(pc p) e -> p pc e", p=P)
                ),
            ],
            "gpsimd": [
                lambda q: q.dma_start(wqkk[:, 0], wqkk_d[0]),
                xd(0, 2), xd(0, 5), xd(0, 6),
                lambda q: q.dma_start(wv[:, 0], wv_d[0]),
                lambda q: q.dma_start(wv[:, 1], wv_d[1]),
                xd(1, 2), xd(1, 5), xd(1, 6),
            ],
        }
        for qname, plan in dma_plan.items():
            q = getattr(nc, qname)
            for fn in plan:
                fn(q)
        ones = persist.tile([P, 1], bf16)
        nc.vector.memset(ones[:], 1.0)
        nc.vector.tensor_copy(
            out=vsb[:, :, :, 64:65],
            in_=ones[:, None, :, None].to_broadcast((P, MC, 4, 1)),
        )

        # PE warm-up: dependency-free matmuls on a zeroed scratch tile flip
        # the HAM clock gate to 8/8 (~3.4us of activity) while the input
        # DMAs stream, so the first projection chain runs at 2.4GHz
        warm = ph1.tile([P, 512], bf16, tag="warm")
        nc.vector.memset(warm[:], 0.0)
        wps = sm_pool.tile([P, 512], fp32, tag="sm")
        for _ in range(12):
            nc.tensor.matmul(wps[:], warm[:, 0:128], warm[:], start=True, stop=True)

        def emit_qk_group(jc, wcol, n5, pool=None):
            # one 512-wide projection group (8 accumulating matmuls + evict)
            if pool is None:
                ps = sm_pool.tile([P, 512], fp32, tag="sm")
            else:
                ps = pool.tile([P, 512], fp32, tag="s")
            wt = wqkq if jc < 2 else wqkk
            wbase = wcol if jc < 2 else wcol - 256
            for cc in range(CC):
                wc = (cc % 4) * 256 + wbase
                nc.tensor.matmul(
                    ps[:],
                    wt[:, cc // 4, wc:wc + 128],
                    xt[:, cc, ts(n5, 512)],
                    start=(cc == 0),
                    stop=(cc == CC - 1),
                )
            if jc < 2:  # q: add bias
                nc.vector.tensor_scalar_add(
                    qk[:, jc, ts(n5, 512)], ps[:], bq[:, jc:jc + 1]
                )
            else:  # k: bias dropped (softmax-invariant)
                nc.vector.tensor_copy(out=qk[:, jc, ts(n5, 512)], in_=ps[:])

        def emit_qk(pc):
            # k(0)/q(0) are jit-emitted inside the first pair's last loop
            # (they gate this pair's first score matmul); the rest follows
            for n5 in range(1, NC5):
                emit_qk_group(2 + pc, 256 + pc * 128, n5)
            for n5 in range(1, NC5):
                emit_qk_group(pc, pc * 128, n5)

        def emit_v_chunk(nt):
            psv = sm_pool.tile([P, 256], fp32, tag="sm")
            for cc in range(CC):
                vc = (cc % 4) * 256
                nc.tensor.matmul(
                    psv[:],
                    xt[:, cc, ts(nt, 128)],
                    wv[:, cc // 4, vc:vc + 256],
                    start=(cc == 0),
                    stop=(cc == CC - 1),
                )
            nc.vector.tensor_copy(
                out=vsb[:, nt, :, 0:64],
                in_=psv[:].rearrange("p (h d) -> p h d", d=64),
            )

        def emit_attn(pc, jit=False):
            # jit=True (first pair): q groups for n5>0 and the V projection
            # are emitted just-in-time inside this loop, so the first exp
            # only waits on k + one q group, and V fills PE slack while the
            # ACT-bound attention stream runs.
            for n5 in range(NC5):
                o2a = o2_pool.tile([65, 512], fp32, tag="o2")
                o2b = o2_pool.tile([65, 512], fp32, tag="o2")
                for mc in range(MC):
                    s = s_pool.tile([P, 1024], fp32, tag="s")
                    nc.tensor.matmul(
                        s[:, 0:512],
                        qk[0:64, 2 + pc, ts(mc, 128)],
                        qk[0:64, pc, ts(n5, 512)],
                        start=True, stop=True, tile_position=(0, 0),
                    )
                    nc.tensor.matmul(
                        s[:, 512:1024],
                        qk[64:128, 2 + pc, ts(mc, 128)],
                        qk[64:128, pc, ts(n5, 512)],
                        start=True, stop=True, tile_position=(64, 0),
                    )
                    pab = p_pool.tile([P, 1024], bf16, tag="pab")
                    nc.scalar.activation(pab[:], s[:], Exp, scale=0.125)
                    if jit and n5 == 0:
                        if mc == 5:
                            # k(3) depends on x token-half 1; emitting it here
                            # keeps its pool slot from gating the first scores
                            emit_qk_group(2, 256, 3)
                        emit_v_chunk(mc)
                    if jit and mc == 10 and n5 < NC5 - 1:
                        # next query group, emitted mid-loop so the next n5's
                        # first exp never stalls on it
                        emit_qk_group(pc, pc * 128, n5 + 1)
                    if jit and n5 == NC5 - 1 and mc == 1:
                        emit_qk_group(3, 384, 0)   # pair 1's k(0)
                    if jit and n5 == NC5 - 1 and mc == 5:
                        emit_qk_group(1, 128, 0)   # pair 1's q(0)
                    nc.tensor.matmul(
                        o2a[:], vsb[:, mc, 2 * pc, 0:65], pab[:, 0:512],
                        start=(mc == 0), stop=(mc == MC - 1),
                    )
                    nc.tensor.matmul(
                        o2b[:], vsb[:, mc, 2 * pc + 1, 0:65], pab[:, 512:1024],
                        start=(mc == 0), stop=(mc == MC - 1),
                    )
                for hl, o2 in ((0, o2a), (1, o2b)):
                    # free the psum bank with one copy; normalize lazily
                    o2s = ev_pool.tile([65, 512], fp32, tag="o2s")
                    nc.vector.tensor_copy(out=o2s[:], in_=o2[:])
                    # partition-broadcast the denominator row via a DRAM
                    # bounce, then reciprocal across all 64 lanes
                    rd = dram_pool.tile([1, 512], fp32, tag="rd")
                    nc.sync.dma_start(rd[:], o2s[64:65, :])
                    rb = ev_pool.tile([64, 512], fp32, tag="rb")
                    rd_bcast = bass.AP(
                        tensor=rd.tensor, offset=rd.offset, ap=[[0, 64], [1, 512]]
                    )
                    nc.sync.dma_start(rb[:], rd_bcast)
                    nc.vector.reciprocal_approx_fast(out=rb[:], in_=rb[:])
                    if hl == 0:
                        dst = o2n[0:64, pc, ts(n5, 512)]
                        nc.vector.scalar_tensor_tensor(
                            dst, o2s[0:64, :], 1.0, rb[:], op0=mult, op1=mult
                        )
                        nc.vector.tensor_scalar_add(
                            dst, dst, bv[:, 2 * pc:2 * pc + 1]
                        )
                    else:
                        stg = ev_pool.tile([64, 512], bf16, tag="stg")
                        nc.vector.scalar_tensor_tensor(
                            stg[:], o2s[0:64, :], 1.0, rb[:], op0=mult, op1=mult
                        )
                        nc.vector.tensor_scalar_add(
                            stg[:], stg[:], bv[:, 2 * pc + 1:2 * pc + 2]
                        )
                        nc.sync.dma_start(o2n[64:128, pc, ts(n5, 512)], stg[:])

        def emit_proj(late):
            for nt in range(MC):
                po = late.tile([P, 1024], bf16, tag="po")
                for ec in range(2):
                    pp = sm_pool.tile([P, 512], fp32, tag="sm")
                    for pc in range(2):
                        nc.tensor.matmul(
                            pp[:], o2n[:, pc, ts(nt, 128)], wp[:, pc, ts(ec, 512)],
                            start=(pc == 0), stop=(pc == 1),
                        )
                    nc.vector.tensor_copy(out=po[:, ts(ec, 512)], in_=pp[:])
                nc.sync.dma_start(out_d[ts(nt, 128), :], po[:])

        # startup: spread the first groups across both psum pools (the
        # scores pool is idle until attention starts) so more groups can
        # stream chunk-by-chunk while x is still loading from HBM; k(3)
        # depends on x token-half 1 and is jit-emitted inside the loop
        # instead, so its pool slot never gates the first score matmuls
        for n5 in range(NC5 - 1):
            emit_qk_group(2, 256, n5, pool=(s_pool if n5 % 2 else None))
        emit_qk_group(0, 0, 0, pool=s_pool)
        emit_attn(0, jit=True)          # jit-emits q n5=1..3 and all of V
        emit_qk(1)
        ph1_cm.__exit__(None, None, None)  # free xt/weights space
        emit_attn(1)
        with tc.tile_pool(name="late", bufs=3) as late:
            emit_proj(late)

    nc.compile()
    return nc


def _get_built(n_tok):
    if n_tok not in _BUILT:
        _BUILT[n_tok] = _build(n_tok)
    return _BUILT[n_tok]


def make_in_map(x_b, w_qkv, b_qkv, w_proj, g):
    """Per-core input shards: batch slice x_b, head-group g (4 heads)."""
    import ml_dtypes

    f = np.float32
    bf = ml_dtypes.bfloat16
    cq = slice(g * 256, g * 256 + 256)
    ck = slice(C + g * 256, C + g * 256 + 256)
    cv = slice(2 * C + g * 256, 2 * C + g * 256 + 256)
    n_tok = x_b.shape[0]

    def quads(w):   # [C, 256] -> [2, 128, 1024] contraction-chunk quads
        return np.ascontiguousarray(
            w.reshape(2, 4, 128, 256).transpose(0, 2, 1, 3).reshape(2, 128, 1024)
        )

    xt = np.asarray(x_b, f).T.astype(bf)            # [C, n_tok]
    xt = xt.reshape(8, 128, n_tok // 1024, 1024).transpose(2, 0, 1, 3)
    return {
        "xt": np.ascontiguousarray(xt),
        "w_qkq": quads(np.asarray(w_qkv[:, cq], f).astype(bf)),
        "w_qkk": quads(np.asarray(w_qkv[:, ck], f).astype(bf)),
        "w_v": quads(np.asarray(w_qkv[:, cv], f).astype(bf)),
        "w_p": np.ascontiguousarray(
            np.asarray(w_proj[g * 256:(g + 1) * 256, :], f).astype(bf)
        ),
        "b_q": np.ascontiguousarray(np.asarray(b_qkv[cq], f)),
        "b_v2": np.ascontiguousarray(np.asarray(b_qkv[cv], f).reshape(4, 64).T),
    }


def kernel(x, w_qkv, b_qkv, w_proj, b_proj):
    from concourse.bass_utils import run_bass_kernel_spmd

    x = np.asarray(x, np.float32)
    B, n_tok, _ = x.shape
    nc = _get_built(n_tok)

    in_maps = [
        make_in_map(x[core // 4], w_qkv, b_qkv, w_proj, core % 4)
        for core in range(N_CORES)
    ]
    res = run_bass_kernel_spmd(
        nc, in_maps, core_ids=list(range(N_CORES)), trace=TRACE
    )
    global LAST_RESULTS
    LAST_RESULTS = res
    outs = [np.asarray(r["out"], np.float32) for r in res.results]
    bp = np.asarray(b_proj, np.float32)
    full = np.stack(
        [
            outs[4 * b] + outs[4 * b + 1] + outs[4 * b + 2] + outs[4 * b + 3] + bp
            for b in range(B)
        ]
    )
    return full.astype(np.float32)
